# revision 55
# baseline (speedup 1.0000x reference)
"""Trainium2 Bass kernel for nn_Dynamics (GNN message passing).

Data-parallel over batch n=1024 across 8 NeuronCores (128 rows each).
All activations on-chip are channel-major: (channels, batch*obj) so every
Linear is a single PE matmul with the stored (fan_in, fan_out) weight as lhsT.

The all-pairs first layer is built by PSUM accumulation of three matmuls per
512-pair block:
    pair1 = Wi.T @ x_i  +  Wj.T @ x_j  +  Wd.T @ dist
where x_i / x_j are step-0 broadcast views of the (32, n*o) state tensor and
dist rows are streamed from a repacked (1, 8192) per-chunk buffer.
rel and att towers are fused along the output-channel (M) dimension.
"""

import numpy as np
from contextlib import ExitStack

import concourse.bass as bass
import concourse.mybir as mybir
import concourse.tile as tile
from concourse import bacc

F32 = mybir.dt.float32
AF = mybir.ActivationFunctionType
ALU = mybir.AluOpType
AX = mybir.AxisListType

N = 1024
NOBJ = 32
CL = 32
NCORES = 8
NP = N // NCORES          # 128 batch rows per core
NO = NP * NOBJ            # 4096 objects per core
NPAIR = NP * NOBJ * NOBJ  # 131072 pairs per core
BLK = 512                 # pairs per PSUM bank (fp32)
NCHUNK = 16               # n-chunks of 8 batch rows
CHUNK_N = NP // NCHUNK    # 8


def _build_program():
    nc = bacc.Bacc("TRN2", target_bir_lowering=False, debug=False)

    def din(name, shape):
        return nc.dram_tensor(name, list(shape), F32, kind="ExternalInput").ap()

    s3d = din("s3d", (NP, NOBJ, 16))
    w_s2 = din("w_s2", (128, 32))      # enc weights replicated on 4 strips
    b_s2 = din("b_s2", (32, 1))
    wi_cat = din("wi_cat", (32, 128))
    wj_cat = din("wj_cat", (32, 128))
    wd_cat = din("wd_cat", (1, 128))
    b_l1 = din("b_l1", (128, 1))
    w1_cat = din("w1_cat", (128, 64))  # blockdiag(rel_w1, att_w1)
    b_l2c = din("b_l2c", (64, 1))      # [rel_b1; att_b1]
    w2_cat = din("w2_cat", (64, 33))   # blockdiag(rel_w2, att_w2)
    b3_relc = din("b3_relc", (33, 1))  # [rel_b2; 0] (bias folded in p3 evac)
    b3_att = din("b3_att", (32, 1))    # att_b2 replicated (exp bias)
    diagneg = din("diagneg", (33, 2 * BLK))  # row 32: -1e30 on diag cols
    ones2b = din("ones2b", (33, 32))   # all-ones at row 32
    negrow = din("negrow", (33, 32))   # row 32 = -1e30 (diag lhsT)
    w_self0 = din("w_self0", (32, 32))
    b_self0 = din("b_self0", (32, 1))
    w_self1 = din("w_self1", (32, 32))
    b_self1 = din("b_self1", (32, 1))
    w_aff0 = din("w_aff0", (32, 32))
    b_aff0 = din("b_aff0", (32, 1))
    w_aff1 = din("w_aff1", (32, 32))
    b_aff1 = din("b_aff1", (32, 1))
    w_aff2 = din("w_aff2", (32, 32))
    b_aff2 = din("b_aff2", (32, 1))
    w_out0 = din("w_out0", (64, 32))
    b_out0 = din("b_out0", (32, 1))
    w_out1 = din("w_out1", (32, 32))
    b_out1 = din("b_out1", (32, 1))
    iden = din("iden", (128, 128))

    out_d = nc.dram_tensor("out", [NP, NOBJ, CL], F32, kind="ExternalOutput").ap()

    with tile.TileContext(nc) as tc, ExitStack() as ctx:
        const = ctx.enter_context(tc.tile_pool(name="const", bufs=1))

        def load_const(ap_in, shape=None, f32r=False):
            nm = f"t_{ap_in.name}"
            t = const.tile(list(shape), F32, name=nm, tag=nm)
            nc.gpsimd.dma_start(out=t, in_=ap_in)
            if f32r:
                tr = const.tile(list(shape), F32R, name=nm + "_r", tag=nm + "_r")
                nc.vector.tensor_copy(tr, t)
                return tr
            return t

        # --- constants to SBUF ---
        t_ws2 = load_const(w_s2, (128, 32))
        t_bs2 = load_const(b_s2, (32, 1))
        t_wi = load_const(wi_cat, f32r=True, shape=(32, 128))
        t_wj = load_const(wj_cat, f32r=True, shape=(32, 128))
        t_wd = load_const(wd_cat, f32r=True, shape=(1, 128))
        t_bl1 = load_const(b_l1, (128, 1))
        t_w1c = load_const(w1_cat, f32r=True, shape=(128, 64))
        t_bl2c = load_const(b_l2c, (64, 1))
        t_w2c = load_const(w2_cat, f32r=True, shape=(64, 33))
        t_b3r = load_const(b3_relc, (33, 1))
        t_b3a = load_const(b3_att, (32, 1))
        t_diag = load_const(diagneg, f32r=True, shape=(33, 2 * BLK))
        t_o2b = load_const(ones2b, f32r=True, shape=(33, 32))
        t_negr = load_const(negrow, f32r=True, shape=(33, 32))
        t_wself0 = load_const(w_self0, f32r=True, shape=(32, 32))
        t_bself0 = load_const(b_self0, (32, 1))
        t_wself1 = load_const(w_self1, f32r=True, shape=(32, 32))
        t_bself1 = load_const(b_self1, (32, 1))
        t_waff0 = load_const(w_aff0, f32r=True, shape=(32, 32))
        t_baff0 = load_const(b_aff0, (32, 1))
        t_waff1 = load_const(w_aff1, f32r=True, shape=(32, 32))
        t_baff1 = load_const(b_aff1, (32, 1))
        t_waff2 = load_const(w_aff2, f32r=True, shape=(32, 32))
        t_baff2 = load_const(b_aff2, (32, 1))
        t_wout0 = load_const(w_out0, f32r=True, shape=(64, 32))
        t_bout0 = load_const(b_out0, (32, 1))
        t_wout1 = load_const(w_out1, f32r=True, shape=(32, 32))
        t_bout1 = load_const(b_out1, (32, 1))
        t_iden = load_const(iden, (128, 128))

        # --- state loads: (n, o, c) padded to c=32 so transposes land 32-aligned ---
        s_pad = const.tile([NP, NOBJ, 32], F32)
        nc.gpsimd.memset(s_pad, 0.0)
        nc.gpsimd.dma_start(out=s_pad[:, :, 0:16], in_=s3d)
        pre_cm = tc.tile_pool(name="pre", bufs=1)
        pre = pre_cm.__enter__()
        pre_ps_cm = tc.tile_pool(name="pre_ps", bufs=2, space="PSUM")
        pre_ps = pre_ps_cm.__enter__()

        # --- s2 = concat(pos, enc[2:]) channel-major, via PE transposes ---
        # transpose chunk k: (n, 4 o's x 32c) -> ((o_l,c), n); then per o one
        # K=16 matmul from strip 32*o_l gives s2[:, (n, o)] columns (stride 32)
        s2 = const.tile([32, NO], F32R)
        s_flat = s_pad.rearrange("p a b -> p (a b)")     # (128, 1024)
        s2_3d = s2.rearrange("p (n o) -> p n o", o=NOBJ)
        for k in range(8):
            pst = pre_ps.tile([128, 128], F32, tag="ps_tr", name="pst")
            nc.tensor.transpose(pst, s_flat[:, 128 * k:128 * (k + 1)], t_iden)
            xt = pre.tile([128, 128], F32, tag="xt", bufs=2, name="xt")
            nc.vector.tensor_copy(xt, pst)
            for ol in range(4):
                o = 4 * k + ol
                ps = pre_ps.tile([32, 128], F32, tag="ps_enc", name="ps_enc")
                nc.tensor.matmul(ps, t_ws2[32 * ol:32 * ol + 16],
                                 xt[32 * ol:32 * ol + 16],
                                 start=True, stop=True,
                                 tile_position=(32 * ol, 0))
                nc.scalar.activation(s2_3d[:, :, o], ps, AF.Identity,
                                     bias=t_bs2)

        # --- pairwise squared distances, n on partitions ---
        pos = s_pad[:, :, 0:2]
        diff = pre.tile([NP, NOBJ, NOBJ, 2], F32)
        nc.vector.tensor_tensor(
            out=diff,
            in0=pos.unsqueeze(2).broadcast_to([NP, NOBJ, NOBJ, 2]),
            in1=pos.unsqueeze(1).broadcast_to([NP, NOBJ, NOBJ, 2]),
            op=ALU.subtract,
        )
        sq = pre.tile([NP, NOBJ, NOBJ, 2], F32)
        nc.vector.tensor_tensor(out=sq, in0=diff, in1=diff, op=ALU.mult)
        dist_t = const.tile([NP, NOBJ, NOBJ], F32)
        nc.vector.tensor_reduce(dist_t, sq, AX.X, ALU.add)
        dist_tr = const.tile([NP, NOBJ, NOBJ], F32R)
        nc.vector.tensor_copy(dist_tr, dist_t)

        # --- self-dynamics (tiny) ---
        h1 = pre.tile([32, NO], F32R)
        for blk in range(NO // BLK):
            ps = tail_ps.tile([32, BLK], F32, tag="ps_small")
            nc.tensor.matmul(ps, t_wself0, s2[:, blk * BLK:(blk + 1) * BLK],
                             start=True, stop=True)
            nc.scalar.activation(h1[:, blk * BLK:(blk + 1) * BLK], ps,
                                 AF.Relu, bias=t_bself0)
        selfd = const.tile([32, NO], F32)
        for blk in range(NO // BLK):
            ps = tail_ps.tile([32, BLK], F32, tag="ps_small")
            nc.tensor.matmul(ps, t_wself1, h1[:, blk * BLK:(blk + 1) * BLK],
                             start=True, stop=True)
            nc.vector.scalar_tensor_tensor(
                out=selfd[:, blk * BLK:(blk + 1) * BLK], in0=ps,
                scalar=t_bself1, in1=h1[:, blk * BLK:(blk + 1) * BLK].bitcast(F32),
                op0=ALU.add, op1=ALU.add)

        pre_ps_cm.__exit__(None, None, None)
        pre_cm.__exit__(None, None, None)

        # --- pair loop: one 512-pair block (nh, ih) per iteration ---
        # All matmuls are float32r with PSUM dst at partition 0 (HW requires it).
        # L2/L3 use block-diagonal fused weights so rel+att run in one matmul.
        rel_dyn_cm = const.tile([32, NO], F32)
        work_cm = tc.tile_pool(name="work", bufs=4)
        work = work_cm.__enter__()
        distp_cm = tc.tile_pool(name="distp", bufs=2)
        distp = distp_cm.__enter__()
        psA_cm = tc.tile_pool(name="psA", bufs=2, space="PSUM")
        psA = psA_cm.__enter__()
        psB_cm = tc.tile_pool(name="psB", bufs=2, space="PSUM")
        psB = psB_cm.__enter__()
        for c in range(NCHUNK):
            dist_f = distp.tile([1, CHUNK_N * 1024], F32R, tag="dist_f")
            nc.sync.dma_start(
                out=dist_f,
                in_=dist_tr[8 * c: 8 * c + 8].rearrange("p a b -> p (a b)"),
            )
            for nl in range(CHUNK_N):
                nh = c * CHUNK_N + nl  # batch row within core
                prodn = work.tile([32, 2, 16, NOBJ], F32, tag="prodn")
                for ih in range(2):
                    # L1: pair1 = Wi.T x_i + Wj.T x_j + Wd.T dist  (128, 512)
                    p1 = psA.tile([128, 16, NOBJ], F32, tag="p1")
                    xi = s2[:, nh * 32 + ih * 16: nh * 32 + ih * 16 + 16]
                    xi = xi.unsqueeze(2).broadcast_to([32, 16, NOBJ])
                    xj = s2[:, nh * 32: nh * 32 + 32]
                    xj = xj.unsqueeze(1).broadcast_to([32, 16, NOBJ])
                    drow = dist_f[0:1,
                                  nl * 1024 + ih * BLK: nl * 1024 + (ih + 1) * BLK]
                    nc.tensor.matmul(p1, t_wi, xi.bitcast(F32R), start=True, stop=False)
                    nc.tensor.matmul(p1, t_wj, xj.bitcast(F32R), start=False, stop=False)
                    nc.tensor.matmul(
                        p1.rearrange("p a b -> p (a b)"), t_wd, drow,
                        start=False, stop=True)
                    f1 = p1.rearrange("p a b -> p (a b)")
                    pair1 = work.tile([128, BLK], F32R, tag="pair1")
                    nc.scalar.activation(pair1, f1, AF.Relu, bias=t_bl1)
                    # L2: one K=128 block-diag matmul -> [rel2(32); att2(32)]
                    p2 = psB.tile([64, BLK], F32, tag="p2")
                    nc.tensor.matmul(p2, t_w1c, pair1, start=True, stop=True)
                    pair2 = work.tile([64, BLK], F32R, tag="pair2")
                    if (2 * nl + ih) % 4 == 0:
                        nc.scalar.activation(pair2, p2, AF.Relu, bias=t_bl2c)
                    else:
                        nc.vector.tensor_scalar(
                            out=pair2, in0=p2, scalar1=t_bl2c, scalar2=0.0,
                            op0=ALU.add, op1=ALU.max)
                    # L3 (+folded residual, +bias in evac): [relf(32); att3(1)]
                    p3 = psB.tile([33, BLK], F32, tag="p3")
                    nc.tensor.matmul(p3, t_w2c, pair2, start=True, stop=True)
                    attE = work.tile([33, BLK], F32R, tag="attE")
                    nc.vector.tensor_scalar(out=attE, in0=p3, scalar1=t_b3r,
                                            scalar2=None, op0=ALU.add)
                    pwp = psB.tile([32, BLK], F32, tag="pwp", bufs=1)
                    nc.tensor.matmul(pwp, t_o2b[32:33], attE[32:33],
                                     start=True, stop=False, tile_position=(32, 0))
                    nc.tensor.matmul(pwp, t_negr[32:33],
                                     t_diag[32:33, ih * BLK:(ih + 1) * BLK],
                                     start=False, stop=True, tile_position=(32, 0))
                    attW = work.tile([32, BLK], F32, tag="attW")
                    nc.scalar.activation(attW, pwp, AF.Exp, bias=t_b3a)
                    # weighted product; reduced once per batch row below
                    nc.gpsimd.tensor_tensor(
                        out=prodn[:, ih],
                        in0=attE[0:32].bitcast(F32).rearrange("p (a b) -> p a b", b=NOBJ),
                        in1=attW.rearrange("p (a b) -> p a b", b=NOBJ),
                        op=ALU.mult)
                nc.vector.tensor_reduce(
                    rel_dyn_cm[:, nh * 32: nh * 32 + 32].rearrange(
                        "p (k b) -> p k b", k=2),
                    prodn, AX.X, ALU.add)

        psB_cm.__exit__(None, None, None)
        psA_cm.__exit__(None, None, None)
        distp_cm.__exit__(None, None, None)
        work_cm.__exit__(None, None, None)

        # --- tail: dyn = selfd + rel_dyn; affector; output head ---
        tailp_cm = tc.tile_pool(name="tailp", bufs=1)
        tailp = tailp_cm.__enter__()
        tail_ps_cm = tc.tile_pool(name="tail_ps", bufs=2, space="PSUM")
        tail_ps = tail_ps_cm.__enter__()
        dyn = tailp.tile([32, NO], F32R)
        nc.vector.tensor_tensor(out=dyn, in0=selfd, in1=rel_dyn_cm, op=ALU.add)

        aff1 = tailp.tile([32, NO], F32R)
        for blk in range(NO // BLK):
            sl = slice(blk * BLK, (blk + 1) * BLK)
            ps = tail_ps.tile([32, BLK], F32, tag="ps_small")
            nc.tensor.matmul(ps, t_waff0, dyn[:, sl], start=True, stop=True)
            nc.scalar.activation(aff1[:, sl], ps, AF.Tanh, bias=t_baff0)
        aff2 = tailp.tile([32, NO], F32R)
        for blk in range(NO // BLK):
            sl = slice(blk * BLK, (blk + 1) * BLK)
            ps = tail_ps.tile([32, BLK], F32, tag="ps_small")
            nc.tensor.matmul(ps, t_waff1, aff1[:, sl], start=True, stop=True)
            tmp = tailp.tile([32, BLK], F32, tag="afftmp", bufs=2, name="tmp")
            nc.scalar.activation(tmp, ps, AF.Tanh, bias=t_baff1)
            nc.vector.tensor_tensor(out=aff2[:, sl], in0=tmp, in1=aff1[:, sl].bitcast(F32),
                                    op=ALU.add)
        stack = tailp.tile([64, NO], F32R)
        for blk in range(NO // BLK):
            sl = slice(blk * BLK, (blk + 1) * BLK)
            ps = tail_ps.tile([32, BLK], F32, tag="ps_small")
            nc.tensor.matmul(ps, t_waff2, aff2[:, sl], start=True, stop=True)
            nc.scalar.activation(stack[0:32, sl], ps, AF.Identity, bias=t_baff2)
        nc.sync.dma_start(out=stack[32:64], in_=s2)

        res_cm = tailp.tile([32, NO], F32)
        for blk in range(NO // BLK):
            sl = slice(blk * BLK, (blk + 1) * BLK)
            ps = tail_ps.tile([32, BLK], F32, tag="ps_small")
            nc.tensor.matmul(ps, t_wout0, stack[:, sl], start=True, stop=True)
            o1 = tailp.tile([32, BLK], F32R, tag="o1", bufs=2, name="o1")
            nc.scalar.activation(o1, ps, AF.Tanh, bias=t_bout0)
            ps2 = tail_ps.tile([32, BLK], F32, tag="ps_small")
            nc.tensor.matmul(ps2, t_wout1, o1, start=True, stop=True)
            nc.vector.scalar_tensor_tensor(
                out=res_cm[:, sl], in0=ps2, scalar=t_bout1, in1=o1.bitcast(F32),
                op0=ALU.add, op1=ALU.add)

        for k in range(NO // 128):
            pst2 = tail_ps.tile([128, 32], F32, tag="ps_tr2", name="pst2")
            nc.tensor.transpose(pst2, res_cm[:, 128 * k:128 * (k + 1)],
                                t_iden[0:32, 0:32])
            o_sb = tailp.tile([128, 32], F32, tag="o_sb", bufs=3, name="o_sb")
            nc.scalar.activation(o_sb, pst2, AF.Copy)
            nc.sync.dma_start(
                out=out_d[4 * k:4 * k + 4].rearrange("a o c -> (a o) c"),
                in_=o_sb)
        tail_ps_cm.__exit__(None, None, None)
        tailp_cm.__exit__(None, None, None)

    nc.compile()
    return nc


_PROG = None


def _get_program():
    global _PROG
    if _PROG is None:
        _PROG = _build_program()
    return _PROG


def _prep_weights(inp):
    g = lambda k: np.asarray(inp[k], dtype=np.float32)
    w_s2_base = g("state_enc_w").copy()
    b_s2 = g("state_enc_b").copy()
    # s2 keeps raw channels 0-1: overwrite first two output cols with identity
    w_s2_base[:, 0:2] = 0.0
    w_s2_base[0, 0] = 1.0
    w_s2_base[1, 1] = 1.0
    b_s2[0:2] = 0.0
    w_s2 = np.zeros((128, 32), np.float32)
    for q in range(4):
        w_s2[32 * q:32 * q + 16] = w_s2_base

    rel_w0, att_w0 = g("rel_w0"), g("att_w0")   # (65, 64) each
    wi_cat = np.concatenate([rel_w0[:32], att_w0[:32]], axis=1)      # (32,128)
    wj_cat = np.concatenate([rel_w0[32:64], att_w0[32:64]], axis=1)  # (32,128)
    wd_cat = np.concatenate([rel_w0[64:65], att_w0[64:65]], axis=1)  # (1,128)
    b_l1 = np.concatenate([g("rel_b0"), g("att_b0")])                # (128,)

    w1_cat = np.zeros((128, 64), np.float32)
    w1_cat[0:64, 0:32] = g("rel_w1")
    w1_cat[64:128, 32:64] = g("att_w1")
    b_l2c = np.concatenate([g("rel_b1"), g("att_b1")])
    w2_cat = np.zeros((64, 33), np.float32)
    w2_cat[0:32, 0:32] = g("rel_w2") + np.eye(32, dtype=np.float32)
    w2_cat[32:64, 32:33] = g("att_w2")
    b3_relc = g("rel_b2")

    diagneg = np.zeros((33, 2 * BLK), np.float32)
    for ih in range(2):
        m = np.zeros((16, 32), np.float32)
        for il in range(16):
            m[il, ih * 16 + il] = 1.0
        diagneg[32, ih * BLK:(ih + 1) * BLK] = m.reshape(-1)
    ones2b = np.zeros((33, 32), np.float32)
    ones2b[32] = 1.0
    negrow = np.zeros((33, 32), np.float32)
    negrow[32] = -80.0

    col = lambda v: np.ascontiguousarray(v.reshape(-1, 1), dtype=np.float32)
    return {
        "w_s2": w_s2, "b_s2": col(b_s2),
        "wi_cat": wi_cat, "wj_cat": wj_cat, "wd_cat": wd_cat,
        "b_l1": col(b_l1), "w1_cat": w1_cat, "b_l2c": col(b_l2c),
        "w2_cat": w2_cat, "b3_relc": col(np.concatenate([g("rel_b2"), [0.0]]).astype(np.float32)),
        "b3_att": np.full((32, 1), float(g("att_b2").reshape(-1)[0]), np.float32),
        "diagneg": diagneg, "ones2b": ones2b,
        "negrow": negrow,
        "w_self0": g("self_w0"), "b_self0": col(g("self_b0")),
        "w_self1": g("self_w1"), "b_self1": col(g("self_b1")),
        "w_aff0": g("aff_w0"), "b_aff0": col(g("aff_b0")),
        "w_aff1": g("aff_w1"), "b_aff1": col(g("aff_b1")),
        "w_aff2": g("aff_w2"), "b_aff2": col(g("aff_b2")),
        "w_out0": g("out_w0"), "b_out0": col(g("out_b0")),
        "w_out1": g("out_w1"), "b_out1": col(g("out_b1")),
        "iden": np.eye(128, dtype=np.float32),
    }


def kernel(**inputs) -> np.ndarray:
    from concourse.bass_utils import run_bass_kernel_spmd

    nc = _get_program()
    weights = _prep_weights(inputs)
    s = np.asarray(inputs["s"], dtype=np.float32)
    in_maps = []
    for core in range(NCORES):
        m = dict(weights)
        m["s3d"] = np.ascontiguousarray(s[core * NP:(core + 1) * NP])
        in_maps.append(m)
    res = run_bass_kernel_spmd(nc, in_maps, list(range(NCORES)))
    out = np.concatenate([res.results[i]["out"] for i in range(NCORES)], axis=0)
    return out.astype(np.float32)


# revision 60
# speedup vs baseline: 1.0514x; 1.0514x over previous
"""Trainium2 Bass kernel for nn_Dynamics (GNN message passing).

Data-parallel over batch n=1024 across 8 NeuronCores (128 rows each).
All activations on-chip are channel-major: (channels, batch*obj) so every
Linear is a single PE matmul with the stored (fan_in, fan_out) weight as lhsT.

The all-pairs first layer is built by PSUM accumulation of three matmuls per
512-pair block:
    pair1 = Wi.T @ x_i  +  Wj.T @ x_j  +  Wd.T @ dist
where x_i / x_j are step-0 broadcast views of the (32, n*o) state tensor and
dist rows are streamed from a repacked (1, 8192) per-chunk buffer.
rel and att towers are fused along the output-channel (M) dimension.
"""

import numpy as np
from contextlib import ExitStack

import concourse.bass as bass
import concourse.mybir as mybir
import concourse.tile as tile
from concourse import bacc

F32 = mybir.dt.float32
AF = mybir.ActivationFunctionType
ALU = mybir.AluOpType
AX = mybir.AxisListType

N = 1024
NOBJ = 32
CL = 32
NCORES = 8
NP = N // NCORES          # 128 batch rows per core
NO = NP * NOBJ            # 4096 objects per core
NPAIR = NP * NOBJ * NOBJ  # 131072 pairs per core
BLK = 512                 # pairs per PSUM bank (fp32)
NCHUNK = 16               # n-chunks of 8 batch rows
CHUNK_N = NP // NCHUNK    # 8


def _build_program():
    nc = bacc.Bacc("TRN2", target_bir_lowering=False, debug=False)

    def din(name, shape):
        return nc.dram_tensor(name, list(shape), F32, kind="ExternalInput").ap()

    s3d = din("s3d", (NP, NOBJ, 16))
    w_s2 = din("w_s2", (128, 32))      # enc weights replicated on 4 strips
    b_s2 = din("b_s2", (32, 1))
    wi_cat = din("wi_cat", (32, 128))
    wj_cat = din("wj_cat", (32, 128))
    wd_cat = din("wd_cat", (1, 128))
    b_l1 = din("b_l1", (128, 1))
    w1_cat = din("w1_cat", (128, 64))  # blockdiag(rel_w1, att_w1)
    b_l2c = din("b_l2c", (64, 1))      # [rel_b1; att_b1]
    w2_cat = din("w2_cat", (64, 33))   # blockdiag(rel_w2, att_w2)
    b3_relc = din("b3_relc", (33, 1))  # [rel_b2; 0] (bias folded in p3 evac)
    b3_att = din("b3_att", (32, 1))    # att_b2 replicated (exp bias)
    diagneg = din("diagneg", (33, 2 * BLK))  # row 32: -1e30 on diag cols
    ones2b = din("ones2b", (33, 32))   # all-ones at row 32
    negrow = din("negrow", (33, 32))   # row 32 = -1e30 (diag lhsT)
    w_self0 = din("w_self0", (32, 32))
    b_self0 = din("b_self0", (32, 1))
    w_self1 = din("w_self1", (32, 32))
    b_self1 = din("b_self1", (32, 1))
    w_aff0 = din("w_aff0", (32, 32))
    b_aff0 = din("b_aff0", (32, 1))
    w_aff1 = din("w_aff1", (32, 32))
    b_aff1 = din("b_aff1", (32, 1))
    w_aff2 = din("w_aff2", (32, 32))
    b_aff2 = din("b_aff2", (32, 1))
    w_out0 = din("w_out0", (64, 32))
    b_out0 = din("b_out0", (32, 1))
    w_out1 = din("w_out1", (32, 32))
    b_out1 = din("b_out1", (32, 1))
    iden = din("iden", (128, 128))

    out_d = nc.dram_tensor("out", [NP, NOBJ, CL], F32, kind="ExternalOutput").ap()

    with tile.TileContext(nc) as tc, ExitStack() as ctx:
        const = ctx.enter_context(tc.tile_pool(name="const", bufs=1))

        def load_const(ap_in, shape=None, f32r=False):
            nm = f"t_{ap_in.name}"
            t = const.tile(list(shape), F32, name=nm, tag=nm)
            nc.gpsimd.dma_start(out=t, in_=ap_in)
            if f32r:
                tr = const.tile(list(shape), F32R, name=nm + "_r", tag=nm + "_r")
                nc.vector.tensor_copy(tr, t)
                return tr
            return t

        # --- constants to SBUF ---
        t_ws2 = load_const(w_s2, (128, 32))
        t_bs2 = load_const(b_s2, (32, 1))
        t_wi = load_const(wi_cat, f32r=True, shape=(32, 128))
        t_wj = load_const(wj_cat, f32r=True, shape=(32, 128))
        t_wd = load_const(wd_cat, f32r=True, shape=(1, 128))
        t_bl1 = load_const(b_l1, (128, 1))
        t_w1c = load_const(w1_cat, f32r=True, shape=(128, 64))
        t_bl2c = load_const(b_l2c, (64, 1))
        t_w2c = load_const(w2_cat, f32r=True, shape=(64, 33))
        t_b3r = load_const(b3_relc, (33, 1))
        t_b3a = load_const(b3_att, (32, 1))
        t_diag = load_const(diagneg, f32r=True, shape=(33, 2 * BLK))
        t_o2b = load_const(ones2b, f32r=True, shape=(33, 32))
        t_negr = load_const(negrow, f32r=True, shape=(33, 32))
        t_wself0 = load_const(w_self0, f32r=True, shape=(32, 32))
        t_bself0 = load_const(b_self0, (32, 1))
        t_wself1 = load_const(w_self1, f32r=True, shape=(32, 32))
        t_bself1 = load_const(b_self1, (32, 1))
        t_waff0 = load_const(w_aff0, f32r=True, shape=(32, 32))
        t_baff0 = load_const(b_aff0, (32, 1))
        t_waff1 = load_const(w_aff1, f32r=True, shape=(32, 32))
        t_baff1 = load_const(b_aff1, (32, 1))
        t_waff2 = load_const(w_aff2, f32r=True, shape=(32, 32))
        t_baff2 = load_const(b_aff2, (32, 1))
        t_wout0 = load_const(w_out0, f32r=True, shape=(64, 32))
        t_bout0 = load_const(b_out0, (32, 1))
        t_wout1 = load_const(w_out1, f32r=True, shape=(32, 32))
        t_bout1 = load_const(b_out1, (32, 1))
        t_iden = load_const(iden, (128, 128))

        # --- state loads: (n, o, c) padded to c=32 so transposes land 32-aligned ---
        s_pad = const.tile([NP, NOBJ, 32], F32)
        nc.gpsimd.memset(s_pad, 0.0)
        nc.gpsimd.dma_start(out=s_pad[:, :, 0:16], in_=s3d)
        pre_cm = tc.tile_pool(name="pre", bufs=1)
        pre = pre_cm.__enter__()
        pre_ps_cm = tc.tile_pool(name="pre_ps", bufs=2, space="PSUM")
        pre_ps = pre_ps_cm.__enter__()

        # --- s2 = concat(pos, enc[2:]) channel-major, via PE transposes ---
        # transpose chunk k: (n, 4 o's x 32c) -> ((o_l,c), n); then per o one
        # K=16 matmul from strip 32*o_l gives s2[:, (n, o)] columns (stride 32)
        s2 = const.tile([32, NO], F32R)
        s_flat = s_pad.rearrange("p a b -> p (a b)")     # (128, 1024)
        s2_3d = s2.rearrange("p (n o) -> p n o", o=NOBJ)
        for k in range(8):
            pst = pre_ps.tile([128, 128], F32, tag="ps_tr", name="pst")
            nc.tensor.transpose(pst, s_flat[:, 128 * k:128 * (k + 1)], t_iden)
            xt = pre.tile([128, 128], F32, tag="xt", bufs=2, name="xt")
            nc.vector.tensor_copy(xt, pst)
            for ol in range(4):
                o = 4 * k + ol
                ps = pre_ps.tile([32, 128], F32, tag="ps_enc", name="ps_enc")
                nc.tensor.matmul(ps, t_ws2[32 * ol:32 * ol + 16],
                                 xt[32 * ol:32 * ol + 16],
                                 start=True, stop=True,
                                 tile_position=(32 * ol, 0))
                nc.scalar.activation(s2_3d[:, :, o], ps, AF.Identity,
                                     bias=t_bs2)

        # --- pairwise squared distances, n on partitions ---
        pos = s_pad[:, :, 0:2]
        diff = pre.tile([NP, NOBJ, NOBJ, 2], F32)
        nc.vector.tensor_tensor(
            out=diff,
            in0=pos.unsqueeze(2).broadcast_to([NP, NOBJ, NOBJ, 2]),
            in1=pos.unsqueeze(1).broadcast_to([NP, NOBJ, NOBJ, 2]),
            op=ALU.subtract,
        )
        sq = pre.tile([NP, NOBJ, NOBJ, 2], F32)
        nc.vector.tensor_tensor(out=sq, in0=diff, in1=diff, op=ALU.mult)
        dist_t = const.tile([NP, NOBJ, NOBJ], F32)
        nc.vector.tensor_reduce(dist_t, sq, AX.X, ALU.add)
        dist_tr = const.tile([NP, NOBJ, NOBJ], F32R)
        nc.vector.tensor_copy(dist_tr, dist_t)

        # --- self-dynamics (tiny) ---
        h1 = pre.tile([32, NO], F32R)
        for blk in range(NO // BLK):
            ps = tail_ps.tile([32, BLK], F32, tag="ps_small")
            nc.tensor.matmul(ps, t_wself0, s2[:, blk * BLK:(blk + 1) * BLK],
                             start=True, stop=True)
            nc.scalar.activation(h1[:, blk * BLK:(blk + 1) * BLK], ps,
                                 AF.Relu, bias=t_bself0)
        selfd = const.tile([32, NO], F32)
        for blk in range(NO // BLK):
            ps = tail_ps.tile([32, BLK], F32, tag="ps_small")
            nc.tensor.matmul(ps, t_wself1, h1[:, blk * BLK:(blk + 1) * BLK],
                             start=True, stop=True)
            nc.vector.scalar_tensor_tensor(
                out=selfd[:, blk * BLK:(blk + 1) * BLK], in0=ps,
                scalar=t_bself1, in1=h1[:, blk * BLK:(blk + 1) * BLK].bitcast(F32),
                op0=ALU.add, op1=ALU.add)

        pre_ps_cm.__exit__(None, None, None)
        pre_cm.__exit__(None, None, None)

        # --- pair loop: one 512-pair block (nh, ih) per iteration ---
        # All matmuls are float32r with PSUM dst at partition 0 (HW requires it).
        # L2/L3 use block-diagonal fused weights so rel+att run in one matmul.
        rel_dyn_cm = const.tile([32, NO], F32)
        work_cm = tc.tile_pool(name="work", bufs=4)
        work = work_cm.__enter__()
        distp_cm = tc.tile_pool(name="distp", bufs=2)
        distp = distp_cm.__enter__()
        psA_cm = tc.tile_pool(name="psA", bufs=2, space="PSUM")
        psA = psA_cm.__enter__()
        psB_cm = tc.tile_pool(name="psB", bufs=2, space="PSUM")
        psB = psB_cm.__enter__()
        for c in range(NCHUNK):
            dist_f = distp.tile([1, CHUNK_N * 1024], F32R, tag="dist_f")
            nc.sync.dma_start(
                out=dist_f,
                in_=dist_tr[8 * c: 8 * c + 8].rearrange("p a b -> p (a b)"),
            )
            for nl in range(CHUNK_N):
                nh = c * CHUNK_N + nl  # batch row within core
                prodn = work.tile([32, 2, 16, NOBJ], F32, tag="prodn")
                for ih in range(2):
                    # L1: pair1 = Wi.T x_i + Wj.T x_j + Wd.T dist  (128, 512)
                    p1 = psA.tile([128, 16, NOBJ], F32, tag="p1")
                    xi = s2[:, nh * 32 + ih * 16: nh * 32 + ih * 16 + 16]
                    xi = xi.unsqueeze(2).broadcast_to([32, 16, NOBJ])
                    xj = s2[:, nh * 32: nh * 32 + 32]
                    xj = xj.unsqueeze(1).broadcast_to([32, 16, NOBJ])
                    drow = dist_f[0:1,
                                  nl * 1024 + ih * BLK: nl * 1024 + (ih + 1) * BLK]
                    nc.tensor.matmul(p1, t_wi, xi.bitcast(F32R), start=True, stop=False)
                    nc.tensor.matmul(p1, t_wj, xj.bitcast(F32R), start=False, stop=False)
                    nc.tensor.matmul(
                        p1.rearrange("p a b -> p (a b)"), t_wd, drow,
                        start=False, stop=True)
                    f1 = p1.rearrange("p a b -> p (a b)")
                    pair1 = work.tile([128, BLK], F32R, tag="pair1")
                    nc.scalar.activation(pair1, f1, AF.Relu, bias=t_bl1)
                    # L2: one K=128 block-diag matmul -> [rel2(32); att2(32)]
                    p2 = psB.tile([64, BLK], F32, tag="p2")
                    nc.tensor.matmul(p2, t_w1c, pair1, start=True, stop=True)
                    pair2 = work.tile([64, BLK], F32R, tag="pair2")
                    if (2 * nl + ih) % 8 == 0:
                        nc.scalar.activation(pair2, p2, AF.Relu, bias=t_bl2c)
                    else:
                        nc.vector.tensor_scalar(
                            out=pair2, in0=p2, scalar1=t_bl2c, scalar2=0.0,
                            op0=ALU.add, op1=ALU.max)
                    # L3 (+folded residual, +bias in evac): [relf(32); att3(1)]
                    p3 = psB.tile([33, BLK], F32, tag="p3")
                    nc.tensor.matmul(p3, t_w2c, pair2, start=True, stop=True)
                    attE = work.tile([33, BLK], F32R, tag="attE")
                    nc.vector.tensor_scalar(out=attE, in0=p3, scalar1=t_b3r,
                                            scalar2=None, op0=ALU.add)
                    pwp = psB.tile([32, BLK], F32, tag="pwp", bufs=1)
                    nc.tensor.matmul(pwp, t_o2b[32:33], attE[32:33],
                                     start=True, stop=False, tile_position=(32, 0))
                    nc.tensor.matmul(pwp, t_negr[32:33],
                                     t_diag[32:33, ih * BLK:(ih + 1) * BLK],
                                     start=False, stop=True, tile_position=(32, 0))
                    attW = work.tile([32, BLK], F32, tag="attW")
                    nc.scalar.activation(attW, pwp, AF.Exp, bias=t_b3a)
                    # weighted product; reduced once per batch row below
                    nc.gpsimd.tensor_tensor(
                        out=prodn[:, ih],
                        in0=attE[0:32].bitcast(F32).rearrange("p (a b) -> p a b", b=NOBJ),
                        in1=attW.rearrange("p (a b) -> p a b", b=NOBJ),
                        op=ALU.mult)
                prodh = work.tile([32, 2, 16, NOBJ // 2], F32, tag="prodh")
                nc.gpsimd.tensor_tensor(
                    out=prodh,
                    in0=prodn.rearrange("p k a (b t) -> p k a b t", t=2)[:, :, :, :, 0],
                    in1=prodn.rearrange("p k a (b t) -> p k a b t", t=2)[:, :, :, :, 1],
                    op=ALU.add)
                nc.vector.tensor_reduce(
                    rel_dyn_cm[:, nh * 32: nh * 32 + 32].rearrange(
                        "p (k b) -> p k b", k=2),
                    prodh, AX.X, ALU.add)

        psB_cm.__exit__(None, None, None)
        psA_cm.__exit__(None, None, None)
        distp_cm.__exit__(None, None, None)
        work_cm.__exit__(None, None, None)

        # --- tail: dyn = selfd + rel_dyn; affector; output head ---
        tailp_cm = tc.tile_pool(name="tailp", bufs=1)
        tailp = tailp_cm.__enter__()
        tail_ps_cm = tc.tile_pool(name="tail_ps", bufs=2, space="PSUM")
        tail_ps = tail_ps_cm.__enter__()
        dyn = tailp.tile([32, NO], F32R)
        nc.vector.tensor_tensor(out=dyn, in0=selfd, in1=rel_dyn_cm, op=ALU.add)

        aff1 = tailp.tile([32, NO], F32R)
        for blk in range(NO // BLK):
            sl = slice(blk * BLK, (blk + 1) * BLK)
            ps = tail_ps.tile([32, BLK], F32, tag="ps_small")
            nc.tensor.matmul(ps, t_waff0, dyn[:, sl], start=True, stop=True)
            nc.scalar.activation(aff1[:, sl], ps, AF.Tanh, bias=t_baff0)
        aff2 = tailp.tile([32, NO], F32R)
        for blk in range(NO // BLK):
            sl = slice(blk * BLK, (blk + 1) * BLK)
            ps = tail_ps.tile([32, BLK], F32, tag="ps_small")
            nc.tensor.matmul(ps, t_waff1, aff1[:, sl], start=True, stop=True)
            tmp = tailp.tile([32, BLK], F32, tag="afftmp", bufs=2, name="tmp")
            nc.scalar.activation(tmp, ps, AF.Tanh, bias=t_baff1)
            nc.vector.tensor_tensor(out=aff2[:, sl], in0=tmp, in1=aff1[:, sl].bitcast(F32),
                                    op=ALU.add)
        stack = tailp.tile([64, NO], F32R)
        for blk in range(NO // BLK):
            sl = slice(blk * BLK, (blk + 1) * BLK)
            ps = tail_ps.tile([32, BLK], F32, tag="ps_small")
            nc.tensor.matmul(ps, t_waff2, aff2[:, sl], start=True, stop=True)
            nc.scalar.activation(stack[0:32, sl], ps, AF.Identity, bias=t_baff2)
        nc.sync.dma_start(out=stack[32:64], in_=s2)

        res_cm = tailp.tile([32, NO], F32)
        for blk in range(NO // BLK):
            sl = slice(blk * BLK, (blk + 1) * BLK)
            ps = tail_ps.tile([32, BLK], F32, tag="ps_small")
            nc.tensor.matmul(ps, t_wout0, stack[:, sl], start=True, stop=True)
            o1 = tailp.tile([32, BLK], F32R, tag="o1", bufs=2, name="o1")
            nc.scalar.activation(o1, ps, AF.Tanh, bias=t_bout0)
            ps2 = tail_ps.tile([32, BLK], F32, tag="ps_small")
            nc.tensor.matmul(ps2, t_wout1, o1, start=True, stop=True)
            nc.vector.scalar_tensor_tensor(
                out=res_cm[:, sl], in0=ps2, scalar=t_bout1, in1=o1.bitcast(F32),
                op0=ALU.add, op1=ALU.add)

        for k in range(NO // 128):
            pst2 = tail_ps.tile([128, 32], F32, tag="ps_tr2", name="pst2")
            nc.tensor.transpose(pst2, res_cm[:, 128 * k:128 * (k + 1)],
                                t_iden[0:32, 0:32])
            o_sb = tailp.tile([128, 32], F32, tag="o_sb", bufs=3, name="o_sb")
            nc.scalar.activation(o_sb, pst2, AF.Copy)
            nc.sync.dma_start(
                out=out_d[4 * k:4 * k + 4].rearrange("a o c -> (a o) c"),
                in_=o_sb)
        tail_ps_cm.__exit__(None, None, None)
        tailp_cm.__exit__(None, None, None)

    nc.compile()
    return nc


_PROG = None


def _get_program():
    global _PROG
    if _PROG is None:
        _PROG = _build_program()
    return _PROG


def _prep_weights(inp):
    g = lambda k: np.asarray(inp[k], dtype=np.float32)
    w_s2_base = g("state_enc_w").copy()
    b_s2 = g("state_enc_b").copy()
    # s2 keeps raw channels 0-1: overwrite first two output cols with identity
    w_s2_base[:, 0:2] = 0.0
    w_s2_base[0, 0] = 1.0
    w_s2_base[1, 1] = 1.0
    b_s2[0:2] = 0.0
    w_s2 = np.zeros((128, 32), np.float32)
    for q in range(4):
        w_s2[32 * q:32 * q + 16] = w_s2_base

    rel_w0, att_w0 = g("rel_w0"), g("att_w0")   # (65, 64) each
    wi_cat = np.concatenate([rel_w0[:32], att_w0[:32]], axis=1)      # (32,128)
    wj_cat = np.concatenate([rel_w0[32:64], att_w0[32:64]], axis=1)  # (32,128)
    wd_cat = np.concatenate([rel_w0[64:65], att_w0[64:65]], axis=1)  # (1,128)
    b_l1 = np.concatenate([g("rel_b0"), g("att_b0")])                # (128,)

    w1_cat = np.zeros((128, 64), np.float32)
    w1_cat[0:64, 0:32] = g("rel_w1")
    w1_cat[64:128, 32:64] = g("att_w1")
    b_l2c = np.concatenate([g("rel_b1"), g("att_b1")])
    w2_cat = np.zeros((64, 33), np.float32)
    w2_cat[0:32, 0:32] = g("rel_w2") + np.eye(32, dtype=np.float32)
    w2_cat[32:64, 32:33] = g("att_w2")
    b3_relc = g("rel_b2")

    diagneg = np.zeros((33, 2 * BLK), np.float32)
    for ih in range(2):
        m = np.zeros((16, 32), np.float32)
        for il in range(16):
            m[il, ih * 16 + il] = 1.0
        diagneg[32, ih * BLK:(ih + 1) * BLK] = m.reshape(-1)
    ones2b = np.zeros((33, 32), np.float32)
    ones2b[32] = 1.0
    negrow = np.zeros((33, 32), np.float32)
    negrow[32] = -80.0

    col = lambda v: np.ascontiguousarray(v.reshape(-1, 1), dtype=np.float32)
    return {
        "w_s2": w_s2, "b_s2": col(b_s2),
        "wi_cat": wi_cat, "wj_cat": wj_cat, "wd_cat": wd_cat,
        "b_l1": col(b_l1), "w1_cat": w1_cat, "b_l2c": col(b_l2c),
        "w2_cat": w2_cat, "b3_relc": col(np.concatenate([g("rel_b2"), [0.0]]).astype(np.float32)),
        "b3_att": np.full((32, 1), float(g("att_b2").reshape(-1)[0]), np.float32),
        "diagneg": diagneg, "ones2b": ones2b,
        "negrow": negrow,
        "w_self0": g("self_w0"), "b_self0": col(g("self_b0")),
        "w_self1": g("self_w1"), "b_self1": col(g("self_b1")),
        "w_aff0": g("aff_w0"), "b_aff0": col(g("aff_b0")),
        "w_aff1": g("aff_w1"), "b_aff1": col(g("aff_b1")),
        "w_aff2": g("aff_w2"), "b_aff2": col(g("aff_b2")),
        "w_out0": g("out_w0"), "b_out0": col(g("out_b0")),
        "w_out1": g("out_w1"), "b_out1": col(g("out_b1")),
        "iden": np.eye(128, dtype=np.float32),
    }


def kernel(**inputs) -> np.ndarray:
    from concourse.bass_utils import run_bass_kernel_spmd

    nc = _get_program()
    weights = _prep_weights(inputs)
    s = np.asarray(inputs["s"], dtype=np.float32)
    in_maps = []
    for core in range(NCORES):
        m = dict(weights)
        m["s3d"] = np.ascontiguousarray(s[core * NP:(core + 1) * NP])
        in_maps.append(m)
    res = run_bass_kernel_spmd(nc, in_maps, list(range(NCORES)))
    out = np.concatenate([res.results[i]["out"] for i in range(NCORES)], axis=0)
    return out.astype(np.float32)


# revision 65
# speedup vs baseline: 1.0574x; 1.0057x over previous
"""Trainium2 Bass kernel for nn_Dynamics (GNN message passing).

Data-parallel over batch n=1024 across 8 NeuronCores (128 rows each).
All activations on-chip are channel-major: (channels, batch*obj) so every
Linear is a single PE matmul with the stored (fan_in, fan_out) weight as lhsT.

The all-pairs first layer is built by PSUM accumulation of three matmuls per
512-pair block:
    pair1 = Wi.T @ x_i  +  Wj.T @ x_j  +  Wd.T @ dist
where x_i / x_j are step-0 broadcast views of the (32, n*o) state tensor and
dist rows are streamed from a repacked (1, 8192) per-chunk buffer.
rel and att towers are fused along the output-channel (M) dimension.
"""

import numpy as np
from contextlib import ExitStack

import concourse.bass as bass
import concourse.mybir as mybir
import concourse.tile as tile
from concourse import bacc

F32 = mybir.dt.float32
AF = mybir.ActivationFunctionType
ALU = mybir.AluOpType
AX = mybir.AxisListType

N = 1024
NOBJ = 32
CL = 32
NCORES = 8
NP = N // NCORES          # 128 batch rows per core
NO = NP * NOBJ            # 4096 objects per core
NPAIR = NP * NOBJ * NOBJ  # 131072 pairs per core
BLK = 512                 # pairs per PSUM bank (fp32)
NCHUNK = 16               # n-chunks of 8 batch rows
CHUNK_N = NP // NCHUNK    # 8


def _build_program():
    nc = bacc.Bacc("TRN2", target_bir_lowering=False, debug=False)

    def din(name, shape):
        return nc.dram_tensor(name, list(shape), F32, kind="ExternalInput").ap()

    s3d = din("s3d", (NP, NOBJ, 16))
    w_s2 = din("w_s2", (128, 32))      # enc weights replicated on 4 strips
    b_s2 = din("b_s2", (32, 1))
    wi_cat = din("wi_cat", (32, 128))
    wj_cat = din("wj_cat", (32, 128))
    wd_cat = din("wd_cat", (1, 128))
    b_l1 = din("b_l1", (128, 1))
    w1_cat = din("w1_cat", (128, 64))  # blockdiag(rel_w1, att_w1)
    b_l2c = din("b_l2c", (64, 1))      # [rel_b1; att_b1]
    w2_cat = din("w2_cat", (64, 33))   # blockdiag(rel_w2, att_w2)
    b3_relc = din("b3_relc", (33, 1))  # [rel_b2; 0] (bias folded in p3 evac)
    b3_att = din("b3_att", (32, 1))    # att_b2 replicated (exp bias)
    diagneg = din("diagneg", (33, 2 * BLK))  # row 32: -1e30 on diag cols
    ones2b = din("ones2b", (33, 32))   # all-ones at row 32
    negrow = din("negrow", (33, 32))   # row 32 = -1e30 (diag lhsT)
    w_self0 = din("w_self0", (32, 32))
    b_self0 = din("b_self0", (32, 1))
    w_self1 = din("w_self1", (32, 32))
    b_self1 = din("b_self1", (32, 1))
    w_aff0 = din("w_aff0", (32, 32))
    b_aff0 = din("b_aff0", (32, 1))
    w_aff1 = din("w_aff1", (32, 32))
    b_aff1 = din("b_aff1", (32, 1))
    w_aff2 = din("w_aff2", (32, 32))
    b_aff2 = din("b_aff2", (32, 1))
    w_out0 = din("w_out0", (64, 32))
    b_out0 = din("b_out0", (32, 1))
    w_out1 = din("w_out1", (32, 32))
    b_out1 = din("b_out1", (32, 1))
    iden = din("iden", (128, 128))

    out_d = nc.dram_tensor("out", [NP, NOBJ, CL], F32, kind="ExternalOutput").ap()

    with tile.TileContext(nc) as tc, ExitStack() as ctx:
        const = ctx.enter_context(tc.tile_pool(name="const", bufs=1))

        def load_const(ap_in, shape=None, f32r=False):
            nm = f"t_{ap_in.name}"
            t = const.tile(list(shape), F32, name=nm, tag=nm)
            nc.gpsimd.dma_start(out=t, in_=ap_in)
            if f32r:
                tr = const.tile(list(shape), F32R, name=nm + "_r", tag=nm + "_r")
                nc.vector.tensor_copy(tr, t)
                return tr
            return t

        # --- constants to SBUF ---
        t_ws2 = load_const(w_s2, (128, 32))
        t_bs2 = load_const(b_s2, (32, 1))
        t_wi = load_const(wi_cat, f32r=True, shape=(32, 128))
        t_wj = load_const(wj_cat, f32r=True, shape=(32, 128))
        t_wd = load_const(wd_cat, f32r=True, shape=(1, 128))
        t_bl1 = load_const(b_l1, (128, 1))
        t_w1c = load_const(w1_cat, f32r=True, shape=(128, 64))
        t_bl2c = load_const(b_l2c, (64, 1))
        t_w2c = load_const(w2_cat, f32r=True, shape=(64, 33))
        t_b3r = load_const(b3_relc, (33, 1))
        t_b3a = load_const(b3_att, (32, 1))
        t_diag = load_const(diagneg, f32r=True, shape=(33, 2 * BLK))
        t_o2b = load_const(ones2b, f32r=True, shape=(33, 32))
        t_negr = load_const(negrow, f32r=True, shape=(33, 32))
        t_wself0 = load_const(w_self0, f32r=True, shape=(32, 32))
        t_bself0 = load_const(b_self0, (32, 1))
        t_wself1 = load_const(w_self1, f32r=True, shape=(32, 32))
        t_bself1 = load_const(b_self1, (32, 1))
        t_waff0 = load_const(w_aff0, f32r=True, shape=(32, 32))
        t_baff0 = load_const(b_aff0, (32, 1))
        t_waff1 = load_const(w_aff1, f32r=True, shape=(32, 32))
        t_baff1 = load_const(b_aff1, (32, 1))
        t_waff2 = load_const(w_aff2, f32r=True, shape=(32, 32))
        t_baff2 = load_const(b_aff2, (32, 1))
        t_wout0 = load_const(w_out0, f32r=True, shape=(64, 32))
        t_bout0 = load_const(b_out0, (32, 1))
        t_wout1 = load_const(w_out1, f32r=True, shape=(32, 32))
        t_bout1 = load_const(b_out1, (32, 1))
        t_iden = load_const(iden, (128, 128))

        # --- state loads: (n, o, c) padded to c=32 so transposes land 32-aligned ---
        s_pad = const.tile([NP, NOBJ, 32], F32)
        nc.gpsimd.memset(s_pad, 0.0)
        nc.gpsimd.dma_start(out=s_pad[:, :, 0:16], in_=s3d)
        pre_cm = tc.tile_pool(name="pre", bufs=1)
        pre = pre_cm.__enter__()
        pre_ps_cm = tc.tile_pool(name="pre_ps", bufs=2, space="PSUM")
        pre_ps = pre_ps_cm.__enter__()

        # --- s2 = concat(pos, enc[2:]) channel-major, via PE transposes ---
        # transpose chunk k: (n, 4 o's x 32c) -> ((o_l,c), n); then per o one
        # K=16 matmul from strip 32*o_l gives s2[:, (n, o)] columns (stride 32)
        s2 = const.tile([32, NO], F32R)
        s_flat = s_pad.rearrange("p a b -> p (a b)")     # (128, 1024)
        s2_3d = s2.rearrange("p (n o) -> p n o", o=NOBJ)
        for k in range(8):
            pst = pre_ps.tile([128, 128], F32, tag="ps_tr", name="pst")
            nc.tensor.transpose(pst, s_flat[:, 128 * k:128 * (k + 1)], t_iden)
            xt = pre.tile([128, 128], F32, tag="xt", bufs=2, name="xt")
            nc.vector.tensor_copy(xt, pst)
            for ol in range(4):
                o = 4 * k + ol
                ps = pre_ps.tile([32, 128], F32, tag="ps_enc", name="ps_enc")
                nc.tensor.matmul(ps, t_ws2[32 * ol:32 * ol + 16],
                                 xt[32 * ol:32 * ol + 16],
                                 start=True, stop=True,
                                 tile_position=(32 * ol, 0))
                nc.scalar.activation(s2_3d[:, :, o], ps, AF.Identity,
                                     bias=t_bs2)

        # --- pairwise squared distances, n on partitions ---
        pos = s_pad[:, :, 0:2]
        diff = pre.tile([NP, NOBJ, NOBJ, 2], F32)
        nc.vector.tensor_tensor(
            out=diff,
            in0=pos.unsqueeze(2).broadcast_to([NP, NOBJ, NOBJ, 2]),
            in1=pos.unsqueeze(1).broadcast_to([NP, NOBJ, NOBJ, 2]),
            op=ALU.subtract,
        )
        sq = pre.tile([NP, NOBJ, NOBJ, 2], F32)
        nc.vector.tensor_tensor(out=sq, in0=diff, in1=diff, op=ALU.mult)
        dist_t = const.tile([NP, NOBJ, NOBJ], F32)
        nc.vector.tensor_reduce(dist_t, sq, AX.X, ALU.add)
        dist_tr = const.tile([NP, NOBJ, NOBJ], F32R)
        nc.vector.tensor_copy(dist_tr, dist_t)

        # --- self-dynamics (tiny) ---
        h1 = pre.tile([32, NO], F32R)
        for blk in range(NO // BLK):
            ps = tail_ps.tile([32, BLK], F32, tag="ps_small")
            nc.tensor.matmul(ps, t_wself0, s2[:, blk * BLK:(blk + 1) * BLK],
                             start=True, stop=True)
            nc.scalar.activation(h1[:, blk * BLK:(blk + 1) * BLK], ps,
                                 AF.Relu, bias=t_bself0)
        selfd = const.tile([32, NO], F32)
        for blk in range(NO // BLK):
            ps = tail_ps.tile([32, BLK], F32, tag="ps_small")
            nc.tensor.matmul(ps, t_wself1, h1[:, blk * BLK:(blk + 1) * BLK],
                             start=True, stop=True)
            nc.vector.scalar_tensor_tensor(
                out=selfd[:, blk * BLK:(blk + 1) * BLK], in0=ps,
                scalar=t_bself1, in1=h1[:, blk * BLK:(blk + 1) * BLK].bitcast(F32),
                op0=ALU.add, op1=ALU.add)

        pre_ps_cm.__exit__(None, None, None)
        pre_cm.__exit__(None, None, None)

        # --- pair loop: one 512-pair block (nh, ih) per iteration ---
        # All matmuls are float32r with PSUM dst at partition 0 (HW requires it).
        # L2/L3 use block-diagonal fused weights so rel+att run in one matmul.
        rel_dyn_cm = const.tile([32, NO], F32)
        work_cm = tc.tile_pool(name="work", bufs=4)
        work = work_cm.__enter__()
        distp_cm = tc.tile_pool(name="distp", bufs=2)
        distp = distp_cm.__enter__()
        psA_cm = tc.tile_pool(name="psA", bufs=2, space="PSUM")
        psA = psA_cm.__enter__()
        psB_cm = tc.tile_pool(name="psB", bufs=2, space="PSUM")
        psB = psB_cm.__enter__()
        for c in range(NCHUNK):
            dist_f = distp.tile([1, CHUNK_N * 1024], F32R, tag="dist_f")
            nc.sync.dma_start(
                out=dist_f,
                in_=dist_tr[8 * c: 8 * c + 8].rearrange("p a b -> p (a b)"),
            )
            for nl in range(CHUNK_N):
                nh = c * CHUNK_N + nl  # batch row within core
                prodn = work.tile([32, 2, 16, NOBJ], F32, tag="prodn")
                for ih in range(2):
                    # L1: pair1 = Wi.T x_i + Wj.T x_j + Wd.T dist  (128, 512)
                    p1 = psA.tile([128, 16, NOBJ], F32, tag="p1")
                    xi = s2[:, nh * 32 + ih * 16: nh * 32 + ih * 16 + 16]
                    xi = xi.unsqueeze(2).broadcast_to([32, 16, NOBJ])
                    xj = s2[:, nh * 32: nh * 32 + 32]
                    xj = xj.unsqueeze(1).broadcast_to([32, 16, NOBJ])
                    drow = dist_f[0:1,
                                  nl * 1024 + ih * BLK: nl * 1024 + (ih + 1) * BLK]
                    nc.tensor.matmul(p1, t_wi, xi.bitcast(F32R), start=True, stop=False)
                    nc.tensor.matmul(p1, t_wj, xj.bitcast(F32R), start=False, stop=False)
                    nc.tensor.matmul(
                        p1.rearrange("p a b -> p (a b)"), t_wd, drow,
                        start=False, stop=True)
                    f1 = p1.rearrange("p a b -> p (a b)")
                    pair1 = work.tile([128, BLK], F32R, tag="pair1")
                    nc.scalar.activation(pair1, f1, AF.Relu, bias=t_bl1)
                    # L2: one K=128 block-diag matmul -> [rel2(32); att2(32)]
                    p2 = psB.tile([64, BLK], F32, tag="p2")
                    nc.tensor.matmul(p2, t_w1c, pair1, start=True, stop=True)
                    pair2 = work.tile([64, BLK], F32R, tag="pair2")
                    nc.vector.tensor_scalar(
                        out=pair2, in0=p2, scalar1=t_bl2c, scalar2=0.0,
                        op0=ALU.add, op1=ALU.max)
                    # L3 (+folded residual, +bias in evac): [relf(32); att3(1)]
                    p3 = psB.tile([33, BLK], F32, tag="p3")
                    nc.tensor.matmul(p3, t_w2c, pair2, start=True, stop=True)
                    attE = work.tile([33, BLK], F32R, tag="attE")
                    nc.vector.tensor_scalar(out=attE, in0=p3, scalar1=t_b3r,
                                            scalar2=None, op0=ALU.add)
                    pwp = psB.tile([32, BLK], F32, tag="pwp", bufs=1)
                    nc.tensor.matmul(pwp, t_o2b[32:33], attE[32:33],
                                     start=True, stop=False, tile_position=(32, 0))
                    nc.tensor.matmul(pwp, t_negr[32:33],
                                     t_diag[32:33, ih * BLK:(ih + 1) * BLK],
                                     start=False, stop=True, tile_position=(32, 0))
                    attW = work.tile([32, BLK], F32, tag="attW")
                    nc.scalar.activation(attW, pwp, AF.Exp, bias=t_b3a)
                    # weighted product; reduced once per batch row below
                    nc.gpsimd.tensor_tensor(
                        out=prodn[:, ih],
                        in0=attE[0:32].bitcast(F32).rearrange("p (a b) -> p a b", b=NOBJ),
                        in1=attW.rearrange("p (a b) -> p a b", b=NOBJ),
                        op=ALU.mult)
                prodh = work.tile([32, 2, 16, NOBJ // 2], F32, tag="prodh")
                nc.gpsimd.tensor_tensor(
                    out=prodh,
                    in0=prodn.rearrange("p k a (b t) -> p k a b t", t=2)[:, :, :, :, 0],
                    in1=prodn.rearrange("p k a (b t) -> p k a b t", t=2)[:, :, :, :, 1],
                    op=ALU.add)
                nc.vector.tensor_reduce(
                    rel_dyn_cm[:, nh * 32: nh * 32 + 32].rearrange(
                        "p (k b) -> p k b", k=2),
                    prodh, AX.X, ALU.add)

        psB_cm.__exit__(None, None, None)
        psA_cm.__exit__(None, None, None)
        distp_cm.__exit__(None, None, None)
        work_cm.__exit__(None, None, None)

        # --- tail: dyn = selfd + rel_dyn; affector; output head ---
        tailp_cm = tc.tile_pool(name="tailp", bufs=1)
        tailp = tailp_cm.__enter__()
        tail_ps_cm = tc.tile_pool(name="tail_ps", bufs=2, space="PSUM")
        tail_ps = tail_ps_cm.__enter__()
        dyn = tailp.tile([32, NO], F32R)
        nc.vector.tensor_tensor(out=dyn, in0=selfd, in1=rel_dyn_cm, op=ALU.add)

        aff1 = tailp.tile([32, NO], F32R)
        for blk in range(NO // BLK):
            sl = slice(blk * BLK, (blk + 1) * BLK)
            ps = tail_ps.tile([32, BLK], F32, tag="ps_small")
            nc.tensor.matmul(ps, t_waff0, dyn[:, sl], start=True, stop=True)
            nc.scalar.activation(aff1[:, sl], ps, AF.Tanh, bias=t_baff0)
        aff2 = tailp.tile([32, NO], F32R)
        for blk in range(NO // BLK):
            sl = slice(blk * BLK, (blk + 1) * BLK)
            ps = tail_ps.tile([32, BLK], F32, tag="ps_small")
            nc.tensor.matmul(ps, t_waff1, aff1[:, sl], start=True, stop=True)
            tmp = tailp.tile([32, BLK], F32, tag="afftmp", bufs=2, name="tmp")
            nc.scalar.activation(tmp, ps, AF.Tanh, bias=t_baff1)
            nc.vector.tensor_tensor(out=aff2[:, sl], in0=tmp, in1=aff1[:, sl].bitcast(F32),
                                    op=ALU.add)
        stack = tailp.tile([64, NO], F32R)
        for blk in range(NO // BLK):
            sl = slice(blk * BLK, (blk + 1) * BLK)
            ps = tail_ps.tile([32, BLK], F32, tag="ps_small")
            nc.tensor.matmul(ps, t_waff2, aff2[:, sl], start=True, stop=True)
            nc.scalar.activation(stack[0:32, sl], ps, AF.Identity, bias=t_baff2)
        nc.sync.dma_start(out=stack[32:64], in_=s2)

        res_cm = tailp.tile([32, NO], F32)
        for blk in range(NO // BLK):
            sl = slice(blk * BLK, (blk + 1) * BLK)
            ps = tail_ps.tile([32, BLK], F32, tag="ps_small")
            nc.tensor.matmul(ps, t_wout0, stack[:, sl], start=True, stop=True)
            o1 = tailp.tile([32, BLK], F32R, tag="o1", bufs=2, name="o1")
            nc.scalar.activation(o1, ps, AF.Tanh, bias=t_bout0)
            ps2 = tail_ps.tile([32, BLK], F32, tag="ps_small")
            nc.tensor.matmul(ps2, t_wout1, o1, start=True, stop=True)
            nc.vector.scalar_tensor_tensor(
                out=res_cm[:, sl], in0=ps2, scalar=t_bout1, in1=o1.bitcast(F32),
                op0=ALU.add, op1=ALU.add)

        for k in range(NO // 128):
            pst2 = tail_ps.tile([128, 32], F32, tag="ps_tr2", name="pst2")
            nc.tensor.transpose(pst2, res_cm[:, 128 * k:128 * (k + 1)],
                                t_iden[0:32, 0:32])
            o_sb = tailp.tile([128, 32], F32, tag="o_sb", bufs=3, name="o_sb")
            nc.scalar.activation(o_sb, pst2, AF.Copy)
            nc.sync.dma_start(
                out=out_d[4 * k:4 * k + 4].rearrange("a o c -> (a o) c"),
                in_=o_sb)
        tail_ps_cm.__exit__(None, None, None)
        tailp_cm.__exit__(None, None, None)

    nc.compile()
    return nc


_PROG = None


def _get_program():
    global _PROG
    if _PROG is None:
        _PROG = _build_program()
    return _PROG


def _prep_weights(inp):
    g = lambda k: np.asarray(inp[k], dtype=np.float32)
    w_s2_base = g("state_enc_w").copy()
    b_s2 = g("state_enc_b").copy()
    # s2 keeps raw channels 0-1: overwrite first two output cols with identity
    w_s2_base[:, 0:2] = 0.0
    w_s2_base[0, 0] = 1.0
    w_s2_base[1, 1] = 1.0
    b_s2[0:2] = 0.0
    w_s2 = np.zeros((128, 32), np.float32)
    for q in range(4):
        w_s2[32 * q:32 * q + 16] = w_s2_base

    rel_w0, att_w0 = g("rel_w0"), g("att_w0")   # (65, 64) each
    wi_cat = np.concatenate([rel_w0[:32], att_w0[:32]], axis=1)      # (32,128)
    wj_cat = np.concatenate([rel_w0[32:64], att_w0[32:64]], axis=1)  # (32,128)
    wd_cat = np.concatenate([rel_w0[64:65], att_w0[64:65]], axis=1)  # (1,128)
    b_l1 = np.concatenate([g("rel_b0"), g("att_b0")])                # (128,)

    w1_cat = np.zeros((128, 64), np.float32)
    w1_cat[0:64, 0:32] = g("rel_w1")
    w1_cat[64:128, 32:64] = g("att_w1")
    b_l2c = np.concatenate([g("rel_b1"), g("att_b1")])
    w2_cat = np.zeros((64, 33), np.float32)
    w2_cat[0:32, 0:32] = g("rel_w2") + np.eye(32, dtype=np.float32)
    w2_cat[32:64, 32:33] = g("att_w2")
    b3_relc = g("rel_b2")

    diagneg = np.zeros((33, 2 * BLK), np.float32)
    for ih in range(2):
        m = np.zeros((16, 32), np.float32)
        for il in range(16):
            m[il, ih * 16 + il] = 1.0
        diagneg[32, ih * BLK:(ih + 1) * BLK] = m.reshape(-1)
    ones2b = np.zeros((33, 32), np.float32)
    ones2b[32] = 1.0
    negrow = np.zeros((33, 32), np.float32)
    negrow[32] = -80.0

    col = lambda v: np.ascontiguousarray(v.reshape(-1, 1), dtype=np.float32)
    return {
        "w_s2": w_s2, "b_s2": col(b_s2),
        "wi_cat": wi_cat, "wj_cat": wj_cat, "wd_cat": wd_cat,
        "b_l1": col(b_l1), "w1_cat": w1_cat, "b_l2c": col(b_l2c),
        "w2_cat": w2_cat, "b3_relc": col(np.concatenate([g("rel_b2"), [0.0]]).astype(np.float32)),
        "b3_att": np.full((32, 1), float(g("att_b2").reshape(-1)[0]), np.float32),
        "diagneg": diagneg, "ones2b": ones2b,
        "negrow": negrow,
        "w_self0": g("self_w0"), "b_self0": col(g("self_b0")),
        "w_self1": g("self_w1"), "b_self1": col(g("self_b1")),
        "w_aff0": g("aff_w0"), "b_aff0": col(g("aff_b0")),
        "w_aff1": g("aff_w1"), "b_aff1": col(g("aff_b1")),
        "w_aff2": g("aff_w2"), "b_aff2": col(g("aff_b2")),
        "w_out0": g("out_w0"), "b_out0": col(g("out_b0")),
        "w_out1": g("out_w1"), "b_out1": col(g("out_b1")),
        "iden": np.eye(128, dtype=np.float32),
    }


def kernel(**inputs) -> np.ndarray:
    from concourse.bass_utils import run_bass_kernel_spmd

    nc = _get_program()
    weights = _prep_weights(inputs)
    s = np.asarray(inputs["s"], dtype=np.float32)
    in_maps = []
    for core in range(NCORES):
        m = dict(weights)
        m["s3d"] = np.ascontiguousarray(s[core * NP:(core + 1) * NP])
        in_maps.append(m)
    res = run_bass_kernel_spmd(nc, in_maps, list(range(NCORES)))
    out = np.concatenate([res.results[i]["out"] for i in range(NCORES)], axis=0)
    return out.astype(np.float32)
